# revision 13
# baseline (speedup 1.0000x reference)
"""Trainium2 Bass kernel for Transformer-XL relative multi-head attention.

Problem: nn_MultiHeadAttn_27290222199184
  T=1024 queries, MEM=1024 memory, C=2048 keys, B=4, DM=1024, N=16 heads, D=64.

Sharding (8 NeuronCores, SPMD — one program, per-core data slices):
  core = 2*b + nh   (b in 0..3 batch, nh in 0..1 head-half)
  Each core computes attention for batch b over its 8 heads (all T rows) and
  emits the partial output projection vec @ W_o[nd_half]  -> [T, DM].
  Host: sums the two half-partials per batch, adds residual h, layernorm.

Device pipeline per core (head pair p = local heads 2p,2p+1 packed on 128
partitions as partition 64*(hh%2)+d):
  - cat/r transposed via PE into [dm, C] half-chunks
  - projections on PE -> kT [pair, 128, C], r_kT, v [C, nd] spilled to DRAM
    scratch; qT kept resident with biases and SCALE pre-applied
  - per head: BD = q2T.T @ r_kT chunks written to a DRAM buffer, re-read
    through a skewed AP (row stride W-1) realizing the rel-shift
    BD_shift[i,j] = BD_raw[i, j-i+(T-1)]
  - S = AC + BD_shift (DVE), P = exp(S) with fused row-sum (ACT accum_out),
    causal-boundary chunk masked with the mask input via copy_predicated
  - P^T via PE transpose straight from score chunks; vecT = v.T @ P^T (PSUM
    accum); 1/denom applied at the PSUM->SBUF epilogue via a DMA-broadcast
    reciprocal row
  - attn_out = vecT.T @ W_o -> out [T, DM]
"""

import sys
from contextlib import ExitStack

if "/opt/trn_rl_repo" not in sys.path:
    sys.path.insert(0, "/opt/trn_rl_repo")

import numpy as np

import concourse.bass as bass
import concourse.bacc as bacc
import concourse.tile as tile
from concourse import mybir

T, MEM, B, DM, N, D = 1024, 1024, 4, 1024, 16, 64
C = MEM + T
NH = N // 2          # heads per core
NP = NH // 2         # head pairs per core
SCALE = 1.0 / D ** 0.5
LN_EPS = 1e-5

BDW = 2560           # bd scratch row width (elements)
NBD = 8              # bd scratch buffers

F32 = mybir.dt.float32
# matmul compute dtype: float32 (exact, 4 cyc/row) or float32r (1 cyc/row)
DT_MM = mybir.dt.float32r
# dtype of the BD DRAM round-trip: float32 or bfloat16
DT_BD = F32

ADD = mybir.AluOpType.add
MULT = mybir.AluOpType.mult


def _cmax(it):
    """last score 512-chunk containing any unmasked element for i-tile it."""
    return (it * 128 + 127 + MEM) // 512


def _mchunks(it):
    """bd m-chunks (512 wide) of real r_k columns read by i-tile it."""
    return [1, 2, 3] if it < 4 else [0, 1, 2, 3]


def build_nc():
    nc = bacc.Bacc("TRN2", target_bir_lowering=False, debug=False)

    io = {}
    io["cat"] = nc.dram_tensor("cat", [C, DM], DT_MM, kind="ExternalInput")
    io["r"] = nc.dram_tensor("r", [C, DM], DT_MM, kind="ExternalInput")
    for w in ("Wq", "Wk", "Wv", "Wr"):
        io[w] = nc.dram_tensor(w, [DM, NH * D], DT_MM, kind="ExternalInput")
    io["Wo"] = nc.dram_tensor("Wo", [NH * D, DM], DT_MM, kind="ExternalInput")
    io["ident"] = nc.dram_tensor("ident", [128, 128], DT_MM, kind="ExternalInput")
    io["rwb_p"] = nc.dram_tensor("rwb_p", [128, NP], F32, kind="ExternalInput")
    io["rrb_p"] = nc.dram_tensor("rrb_p", [128, NP], F32, kind="ExternalInput")
    io["masku8"] = nc.dram_tensor("masku8", [T, C], mybir.dt.uint8, kind="ExternalInput")
    io["out"] = nc.dram_tensor("out", [T, DM], F32, kind="ExternalOutput")

    io["kT_s"] = nc.dram_tensor("kT_s", [NP, 128, C], DT_MM)
    io["rk_s"] = nc.dram_tensor("rk_s", [NP, 128, C], DT_MM)
    io["v_s"] = nc.dram_tensor("v_s", [C, NH * D], DT_MM)
    io["recip_s"] = nc.dram_tensor("recip_s", [NH, T], F32)
    io["bd"] = [nc.dram_tensor(f"bd_s{i}", [128, BDW], DT_BD) for i in range(NBD)]

    with tile.TileContext(nc) as tc:
        _emit(nc, tc, io)
    nc.compile()
    return nc


def _emit(nc, tc, io):
    ctx = ExitStack()
    with ctx:
        singles = ctx.enter_context(tc.tile_pool(name="singles", bufs=1))
        resid = ctx.enter_context(tc.tile_pool(name="resid", bufs=1))
        catT_p = ctx.enter_context(tc.tile_pool(name="catT", bufs=1))
        wset_p = ctx.enter_context(tc.tile_pool(name="wset", bufs=2))
        rows_p = ctx.enter_context(tc.tile_pool(name="rows", bufs=6))
        st_p = ctx.enter_context(tc.tile_pool(name="st", bufs=4))
        kpair_p = ctx.enter_context(tc.tile_pool(name="kpair", bufs=1))
        vhead_p = ctx.enter_context(tc.tile_pool(name="vhead", bufs=2))
        pch_p = ctx.enter_context(tc.tile_pool(name="pch", bufs=3))
        sch_p = ctx.enter_context(tc.tile_pool(name="sch", bufs=2))
        skew_p = ctx.enter_context(tc.tile_pool(name="skew", bufs=3))
        big_p = ctx.enter_context(tc.tile_pool(name="big", bufs=1))
        mask_p = ctx.enter_context(tc.tile_pool(name="mask", bufs=2))
        den_p = ctx.enter_context(tc.tile_pool(name="den", bufs=3))
        rb_p = ctx.enter_context(tc.tile_pool(name="rb", bufs=2))
        wo_p = ctx.enter_context(tc.tile_pool(name="wo", bufs=2))

        psum_mm = ctx.enter_context(tc.tile_pool(name="psum_mm", bufs=4, space="PSUM"))
        psum_tp = ctx.enter_context(tc.tile_pool(name="psum_tp", bufs=2, space="PSUM"))
        psum_av = ctx.enter_context(tc.tile_pool(name="psum_av", bufs=2, space="PSUM"))

        # ---------------- constants ----------------
        ident = singles.tile([128, 128], DT_MM)
        nc.sync.dma_start(ident, io["ident"].ap())
        neg_t = singles.tile([128, 512], F32)
        nc.vector.memset(neg_t, -70000.0)
        rwb_t = singles.tile([128, NP], F32)
        nc.sync.dma_start(rwb_t, io["rwb_p"].ap())
        rrb_t = singles.tile([128, NP], F32)
        nc.sync.dma_start(rrb_t, io["rrb_p"].ap())

        qbT = resid.tile([128, NP, T], DT_MM)
        q2T = resid.tile([128, NP, T], DT_MM)
        vecT = resid.tile([128, NP, T], DT_MM)

        # bd tails [2048, BDW) are read by boundary chunks (always masked
        # positions) but never written by the BD pass: zero them once.
        zero_bd = singles.tile([128, 512], DT_BD)
        nc.vector.memset(zero_bd, 0.0)
        for buf in io["bd"]:
            nc.sync.dma_start(buf.ap()[:, 2048:2560], zero_bd)

        # ------------- phase A: transposes + projections -------------
        def transpose_half(src, half):
            """src [C, DM] rows half*1024..+1024 -> [128(dm), 8(dmc), 1024(C)]."""
            xT = catT_p.tile([128, 8, 1024], DT_MM, tag="catT")
            for ctg in range(2):          # 512-row groups within the half
                for dmh in range(2):      # 512-col (dm) halves
                    rtiles = []
                    for ct in range(4):
                        row = rows_p.tile([128, 512], DT_MM, tag="rows")
                        r0 = half * 1024 + ctg * 512 + ct * 128
                        nc.sync.dma_start(
                            row, src.ap()[r0:r0 + 128, dmh * 512:(dmh + 1) * 512])
                        rtiles.append(row)
                    for dml in range(4):
                        dmc = dmh * 4 + dml
                        ps = psum_tp.tile([128, 512], DT_MM, tag="tp")
                        for ct in range(4):
                            nc.tensor.transpose(
                                (ps[:, ct * 128:(ct + 1) * 128]),
                                (rtiles[ct][:, dml * 128:(dml + 1) * 128]),
                                (ident),
                            )
                        nc.scalar.copy(xT[:, dmc, ctg * 512:(ctg + 1) * 512], ps)
            return xT

        def load_wset(wname, p):
            ws = wset_p.tile([128, 8, 128], DT_MM, tag="wset")
            nc.sync.dma_start(
                ws,
                io[wname].ap()[:, p * 128:(p + 1) * 128].rearrange(
                    "(o pp) n -> pp o n", pp=128),
            )
            return ws

        wv_t = big_p.tile([128, 8, 512], DT_MM, tag="big")
        nc.sync.dma_start(wv_t, io["Wv"].ap().rearrange("(o pp) n -> pp o n", pp=128))

        for half in range(2):
            catT = transpose_half(io["cat"], half)
            # kT
            for p in range(NP):
                ws = load_wset("Wk", p)
                for ch in range(2):
                    cchunk = half * 2 + ch
                    ps = psum_mm.tile([128, 512], F32, tag="mm")
                    for dmc in range(8):
                        nc.tensor.matmul(
                            ps, (ws[:, dmc, :]), (catT[:, dmc, ch * 512:(ch + 1) * 512]),
                            start=(dmc == 0), stop=(dmc == 7),
                        )
                    st = st_p.tile([128, 512], DT_MM, tag="st")
                    nc.scalar.copy(st, ps)
                    nc.sync.dma_start(
                        io["kT_s"].ap()[p, :, cchunk * 512:(cchunk + 1) * 512], st)
            # v
            for cc in range(8):
                ps = psum_mm.tile([128, 512], F32, tag="mm")
                for dmc in range(8):
                    nc.tensor.matmul(
                        ps, (catT[:, dmc, cc * 128:(cc + 1) * 128]), (wv_t[:, dmc, :]),
                        start=(dmc == 0), stop=(dmc == 7),
                    )
                st = st_p.tile([128, 512], DT_MM, tag="st")
                nc.scalar.copy(st, ps)
                nc.sync.dma_start(
                    io["v_s"].ap()[half * 1024 + cc * 128: half * 1024 + (cc + 1) * 128, :], st)
            # q (cat columns >= MEM live in half 1)
            if half == 1:
                for p in range(NP):
                    ws = load_wset("Wq", p)
                    for ih in range(2):
                        ps = psum_mm.tile([128, 512], F32, tag="mm")
                        for dmc in range(8):
                            nc.tensor.matmul(
                                ps, (ws[:, dmc, :]), (catT[:, dmc, ih * 512:(ih + 1) * 512]),
                                start=(dmc == 0), stop=(dmc == 7),
                            )
                        nc.vector.tensor_scalar(
                            qbT[:, p, ih * 512:(ih + 1) * 512], ps,
                            rwb_t[:, p:p + 1], SCALE, ADD, MULT)
                        nc.vector.tensor_scalar(
                            q2T[:, p, ih * 512:(ih + 1) * 512], ps,
                            rrb_t[:, p:p + 1], SCALE, ADD, MULT)

        for half in range(2):
            rT = transpose_half(io["r"], half)
            for p in range(NP):
                ws = load_wset("Wr", p)
                for ch in range(2):
                    cchunk = half * 2 + ch
                    ps = psum_mm.tile([128, 512], F32, tag="mm")
                    for dmc in range(8):
                        nc.tensor.matmul(
                            ps, (ws[:, dmc, :]), (rT[:, dmc, ch * 512:(ch + 1) * 512]),
                            start=(dmc == 0), stop=(dmc == 7),
                        )
                    st = st_p.tile([128, 512], DT_MM, tag="st")
                    nc.scalar.copy(st, ps)
                    nc.sync.dma_start(
                        io["rk_s"].ap()[p, :, cchunk * 512:(cchunk + 1) * 512], st)

        # ------------- phase B: attention -------------
        for p in range(NP):
            kT_t = kpair_p.tile([128, C], DT_MM, tag="kT")
            nc.sync.dma_start(kT_t, io["kT_s"].ap()[p])
            rk_t = kpair_p.tile([128, C], DT_MM, tag="rk")
            nc.sync.dma_start(rk_t, io["rk_s"].ap()[p])
            for sub in range(2):
                hh = 2 * p + sub
                lo, hi = 64 * sub, 64 * sub + 64
                v_t = vhead_p.tile([128, 16, 64], DT_MM, tag="vhead")
                nc.sync.dma_start(
                    v_t,
                    io["v_s"].ap()[:, hh * 64:(hh + 1) * 64].rearrange(
                        "(cc pp) d -> pp cc d", pp=128),
                )

                # BD pass
                for it in range(8):
                    buf = io["bd"][(hh * 8 + it) % NBD]
                    for a in _mchunks(it):
                        ps = psum_mm.tile([128, 512], F32, tag="mm")
                        nc.tensor.matmul(
                            ps,
                            (q2T[lo:hi, p, it * 128:(it + 1) * 128]),
                            (rk_t[lo:hi, a * 512:(a + 1) * 512]),
                            start=True, stop=True,
                        )
                        st = st_p.tile([128, 512], DT_BD, tag="bdst")
                        nc.scalar.copy(st, ps)
                        nc.sync.dma_start(buf.ap()[:, a * 512:(a + 1) * 512], st)

                denoms = den_p.tile([128, 8, 4], F32, tag="denoms")
                recips = den_p.tile([128, 8], F32, tag="recips")

                # scores -> exp -> P^T, per i-half
                for ihalf in range(2):
                    njc = 12 if ihalf == 0 else 16
                    PT = big_p.tile([128, 16, 512], DT_MM, tag="big")
                    for itl in range(4):
                        it = ihalf * 4 + itl
                        buf = io["bd"][(hh * 8 + it) % NBD]
                        cm = _cmax(it)
                        for c in range(cm + 1):
                            ps = psum_mm.tile([128, 512], F32, tag="mm")
                            nc.tensor.matmul(
                                ps,
                                (qbT[lo:hi, p, it * 128:(it + 1) * 128]),
                                (kT_t[lo:hi, c * 512:(c + 1) * 512]),
                                start=True, stop=True,
                            )
                            skew = skew_p.tile([128, 512], DT_BD, tag="skew")
                            nc.sync.dma_start(
                                skew,
                                bass.AP(buf, 512 * c + (T - 1) - it * 128,
                                        [[BDW - 1, 128], [1, 512]]),
                            )
                            s_t = sch_p.tile([128, 512], F32, tag="S")
                            nc.vector.tensor_tensor(s_t, ps, skew, ADD)
                            if c == cm:
                                # boundary chunk: push masked scores to -inf
                                mk = mask_p.tile([128, 512], mybir.dt.uint8, tag="mask")
                                nc.sync.dma_start(
                                    mk, io["masku8"].ap()[
                                        it * 128:(it + 1) * 128, cm * 512:(cm + 1) * 512])
                                nc.vector.copy_predicated(s_t, mk, neg_t)
                            P_c = pch_p.tile([128, 512], DT_MM, tag="P")
                            nc.scalar.activation(
                                P_c, s_t, mybir.ActivationFunctionType.Exp,
                                accum_out=denoms[:, it, c:c + 1],
                            )
                            # transpose the 4 jc blocks of this chunk into PT
                            tps = psum_tp.tile([128, 512], DT_MM, tag="tp")
                            for j4 in range(4):
                                nc.tensor.transpose(
                                    (tps[:, j4 * 128:(j4 + 1) * 128]),
                                    (P_c[:, j4 * 128:(j4 + 1) * 128]),
                                    (ident),
                                )
                            dst = PT[:, c * 4:(c + 1) * 4, itl * 128:(itl + 1) * 128]
                            src = tps.rearrange("p (a b) -> p a b", a=4)
                            if it % 2 == 0:
                                nc.scalar.copy(dst, src)
                            else:
                                nc.vector.tensor_copy(dst, src)
                        nc.vector.tensor_reduce(
                            recips[:, it:it + 1], denoms[:, it, 0:cm + 1],
                            axis=mybir.AxisListType.X, op=ADD,
                        )
                    # reciprocals for this i-half -> DRAM (re-read broadcast below)
                    hsl = slice(ihalf * 4, (ihalf + 1) * 4)
                    nc.vector.reciprocal(recips[:, hsl], recips[:, hsl])
                    nc.sync.dma_start(
                        bass.AP(io["recip_s"], hh * T + ihalf * 512, [[1, 128], [128, 4]]),
                        recips[:, hsl])
                    av = psum_av.tile([64, 512], F32, tag="av")
                    for jc in range(njc):
                        nc.tensor.matmul(
                            av,
                            (v_t[:, jc, :]),
                            (PT[:, jc, :]),
                            start=(jc == 0), stop=(jc == njc - 1),
                        )
                    rb = rb_p.tile([64, 512], F32, tag="rb")
                    nc.sync.dma_start(
                        rb,
                        bass.AP(io["recip_s"], hh * T + ihalf * 512, [[0, 64], [1, 512]]))
                    if sub == 0:
                        nc.vector.tensor_tensor(
                            vecT[0:64, p, ihalf * 512:(ihalf + 1) * 512], av, rb, MULT)
                    else:
                        # odd head: epilogue at base 0, partition-shift via DMA
                        tmp = rb_p.tile([64, 512], DT_MM, tag="avtmp")
                        nc.vector.tensor_tensor(tmp, av, rb, MULT)
                        nc.sync.dma_start(
                            vecT[64:128, p, ihalf * 512:(ihalf + 1) * 512], tmp)

        # ------------- phase C: output projection -------------
        for dmc in range(2):
            for itg in range(2):
                pss = [psum_mm.tile([128, 512], F32, tag="mm", name=f"wo_ps{i}")
                       for i in range(4)]
                for pp in range(NP):
                    wt = wo_p.tile([128, 512], DT_MM, tag="wo")
                    nc.sync.dma_start(
                        wt, io["Wo"].ap()[pp * 128:(pp + 1) * 128, dmc * 512:(dmc + 1) * 512])
                    for itl in range(4):
                        it = itg * 4 + itl
                        nc.tensor.matmul(
                            pss[itl], (vecT[:, pp, it * 128:(it + 1) * 128]), (wt),
                            start=(pp == 0), stop=(pp == NP - 1),
                        )
                for itl in range(4):
                    it = itg * 4 + itl
                    st = st_p.tile([128, 512], F32, tag="st")
                    nc.scalar.copy(st, pss[itl])
                    nc.sync.dma_start(
                        io["out"].ap()[it * 128:(it + 1) * 128, dmc * 512:(dmc + 1) * 512], st)


_NC = None


def _get_nc():
    global _NC
    if _NC is None:
        _NC = build_nc()
    return _NC


def make_in_maps(h, m, r, mask, W_qkv, W_r, W_o, r_w_bias, r_r_bias):
    h = np.ascontiguousarray(np.asarray(h, dtype=np.float32))
    m = np.ascontiguousarray(np.asarray(m, dtype=np.float32))
    r = np.ascontiguousarray(np.asarray(r, dtype=np.float32))
    masku8 = np.ascontiguousarray(np.asarray(mask).reshape(T, C).astype(np.uint8))
    W_qkv = np.asarray(W_qkv, dtype=np.float32)
    W_r = np.asarray(W_r, dtype=np.float32)
    W_o = np.asarray(W_o, dtype=np.float32)
    rwb = np.asarray(r_w_bias, dtype=np.float32)
    rrb = np.asarray(r_r_bias, dtype=np.float32)

    in_maps = []
    for core in range(8):
        b, nh = core // 2, core % 2
        sl = slice(nh * NH * D, (nh + 1) * NH * D)
        rwb_p = np.zeros((128, NP), np.float32)
        rrb_p = np.zeros((128, NP), np.float32)
        for hh in range(NH):
            g = nh * NH + hh
            rwb_p[64 * (hh % 2):64 * (hh % 2) + 64, hh // 2] = rwb[g]
            rrb_p[64 * (hh % 2):64 * (hh % 2) + 64, hh // 2] = rrb[g]
        in_maps.append({
            "cat": np.ascontiguousarray(np.concatenate([m[:, b, :], h[:, b, :]], axis=0)),
            "r": r,
            "Wq": np.ascontiguousarray(W_qkv[:, 0 * N * D:1 * N * D][:, sl]),
            "Wk": np.ascontiguousarray(W_qkv[:, 1 * N * D:2 * N * D][:, sl]),
            "Wv": np.ascontiguousarray(W_qkv[:, 2 * N * D:3 * N * D][:, sl]),
            "Wr": np.ascontiguousarray(W_r[:, sl]),
            "Wo": np.ascontiguousarray(W_o[sl, :]),
            "rwb_p": rwb_p,
            "rrb_p": rrb_p,
            "masku8": masku8,
            "ident": np.eye(128, dtype=np.float32),
        })
    return in_maps


def finish(h, parts, ln_gamma, ln_beta):
    h = np.asarray(h, dtype=np.float32)
    gamma = np.asarray(ln_gamma, dtype=np.float32)
    beta = np.asarray(ln_beta, dtype=np.float32)
    out = np.empty((T, B, DM), np.float32)
    for b in range(B):
        x = h[:, b, :] + parts[2 * b] + parts[2 * b + 1]
        mu = x.mean(axis=-1, keepdims=True, dtype=np.float32)
        var = ((x - mu) ** 2).mean(axis=-1, keepdims=True, dtype=np.float32)
        out[:, b, :] = (x - mu) / np.sqrt(var + LN_EPS) * gamma + beta
    return out


def kernel(h, m, r, mask, W_qkv, W_r, W_o, r_w_bias, r_r_bias, ln_gamma, ln_beta):
    from concourse.bass_utils import run_bass_kernel_spmd

    in_maps = make_in_maps(h, m, r, mask, W_qkv, W_r, W_o, r_w_bias, r_r_bias)
    res = run_bass_kernel_spmd(_get_nc(), in_maps, core_ids=list(range(8)))
    parts = [np.asarray(res.results[c]["out"]) for c in range(8)]
    return finish(h, parts, ln_gamma, ln_beta)


# revision 34
# speedup vs baseline: 1.1413x; 1.1413x over previous
"""Trainium2 Bass kernel for Transformer-XL relative multi-head attention.

Problem: nn_MultiHeadAttn_27290222199184
  T=1024 queries, MEM=1024 memory, C=2048 keys, B=4, DM=1024, N=16 heads, D=64.

Sharding (8 NeuronCores, SPMD — one program, per-core data slices):
  core = 2*b + nh   (b in 0..3 batch, nh in 0..1 head-half)
  Each core computes attention for batch b over its 8 heads (all T rows) and
  emits the partial output projection vec @ W_o[nd_half]  -> [T, DM].
  Host: sums the two half-partials per batch, adds residual h, layernorm.

Device pipeline per core (head pair p = local heads 2p,2p+1 packed on 128
partitions as partition 64*(hh%2)+d):
  - cat/r transposed via PE into [dm, C] half-chunks
  - projections on PE -> kT [pair, 128, C], r_kT, v [C, nd] spilled to DRAM
    scratch; qT kept resident with biases and SCALE pre-applied
  - per head: BD = q2T.T @ r_kT chunks written to a DRAM buffer, re-read
    through a skewed AP (row stride W-1) realizing the rel-shift
    BD_shift[i,j] = BD_raw[i, j-i+(T-1)]
  - S = AC + BD_shift (DVE), P = exp(S) with fused row-sum (ACT accum_out),
    causal-boundary chunk masked with the mask input via copy_predicated
  - P^T via PE transpose straight from score chunks; vecT = v.T @ P^T (PSUM
    accum); 1/denom applied at the PSUM->SBUF epilogue via a DMA-broadcast
    reciprocal row
  - attn_out = vecT.T @ W_o -> out [T, DM]
"""

import sys
from contextlib import ExitStack

if "/opt/trn_rl_repo" not in sys.path:
    sys.path.insert(0, "/opt/trn_rl_repo")

import numpy as np

import concourse.bass as bass
import concourse.bacc as bacc
import concourse.tile as tile
from concourse import mybir

T, MEM, B, DM, N, D = 1024, 1024, 4, 1024, 16, 64
C = MEM + T
NH = N // 2          # heads per core
NP = NH // 2         # head pairs per core
SCALE = 1.0 / D ** 0.5
LN_EPS = 1e-5

BDW = 2560           # bd scratch row width (elements)
NBD = 16             # bd scratch buffers

F32 = mybir.dt.float32
# matmul compute dtype: float32 (exact, 4 cyc/row) or float32r (1 cyc/row)
DT_MM = mybir.dt.float32r
# dtype of the BD DRAM round-trip: float32 or bfloat16
DT_BD = F32

ADD = mybir.AluOpType.add
MULT = mybir.AluOpType.mult


def _cmax(it):
    """last score 512-chunk containing any unmasked element for i-tile it."""
    return (it * 128 + 127 + MEM) // 512


def _mchunks(it):
    """bd m-chunks (512 wide) of real r_k columns read by i-tile it."""
    return [1, 2, 3] if it < 4 else [0, 1, 2, 3]


def _mlo(it):
    """first bd column read by i-tile it (skew-read window start)."""
    return max(0, (T - 1) - it * 128 - 127)


def _wb(it):
    """boundary-chunk read width: last unmasked col within chunk cmax + 1."""
    return it * 128 + 127 + MEM - 512 * _cmax(it) + 1


def build_nc():
    nc = bacc.Bacc("TRN2", target_bir_lowering=False, debug=False)

    io = {}
    io["cat"] = nc.dram_tensor("cat", [C, DM], DT_MM, kind="ExternalInput")
    io["r"] = nc.dram_tensor("r", [C, DM], DT_MM, kind="ExternalInput")
    for w in ("Wq", "Wk", "Wv", "Wr"):
        io[w] = nc.dram_tensor(w, [DM, NH * D], DT_MM, kind="ExternalInput")
    io["Wo"] = nc.dram_tensor("Wo", [NH * D, DM], DT_MM, kind="ExternalInput")
    io["ident"] = nc.dram_tensor("ident", [128, 128], DT_MM, kind="ExternalInput")
    io["rwb_p"] = nc.dram_tensor("rwb_p", [128, NP], F32, kind="ExternalInput")
    io["rrb_p"] = nc.dram_tensor("rrb_p", [128, NP], F32, kind="ExternalInput")
    io["masku8"] = nc.dram_tensor("masku8", [T, C], mybir.dt.uint8, kind="ExternalInput")
    io["out"] = nc.dram_tensor("out", [T, DM], F32, kind="ExternalOutput")

    io["kT_s"] = nc.dram_tensor("kT_s", [NP, 128, C], DT_MM)
    io["rk_s"] = nc.dram_tensor("rk_s", [NP, 128, C], DT_MM)
    io["v_s"] = nc.dram_tensor("v_s", [C, NH * D], DT_MM)
    io["recip_s"] = nc.dram_tensor("recip_s", [NH, T], F32)
    io["bd"] = [nc.dram_tensor(f"bd_s{i}", [128, BDW], DT_BD) for i in range(NBD)]

    with tile.TileContext(nc) as tc:
        _emit(nc, tc, io)
    nc.compile()
    return nc


def _emit(nc, tc, io):
    ctx = ExitStack()
    with ctx:
        singles = ctx.enter_context(tc.tile_pool(name="singles", bufs=1))
        resid = ctx.enter_context(tc.tile_pool(name="resid", bufs=1))
        catT_p = ctx.enter_context(tc.tile_pool(name="catT", bufs=1))
        wset_p = ctx.enter_context(tc.tile_pool(name="wset", bufs=2))
        rows_p = ctx.enter_context(tc.tile_pool(name="rows", bufs=5))
        st_p = ctx.enter_context(tc.tile_pool(name="st", bufs=4))
        kpair_p = ctx.enter_context(tc.tile_pool(name="kpair", bufs=1))
        vhead_p = ctx.enter_context(tc.tile_pool(name="vhead", bufs=2))
        pch_p = ctx.enter_context(tc.tile_pool(name="pch", bufs=3))
        sch_p = ctx.enter_context(tc.tile_pool(name="sch", bufs=2))
        skew_p = ctx.enter_context(tc.tile_pool(name="skew", bufs=4))
        big_p = ctx.enter_context(tc.tile_pool(name="big", bufs=1))
        mask_p = ctx.enter_context(tc.tile_pool(name="mask", bufs=2))
        den_p = ctx.enter_context(tc.tile_pool(name="den", bufs=3))
        rb_p = ctx.enter_context(tc.tile_pool(name="rb", bufs=2))
        wo_p = ctx.enter_context(tc.tile_pool(name="wo", bufs=2))

        psum_mm = ctx.enter_context(tc.tile_pool(name="psum_mm", bufs=5, space="PSUM"))
        psum_tp = ctx.enter_context(tc.tile_pool(name="psum_tp", bufs=2, space="PSUM"))
        psum_av = ctx.enter_context(tc.tile_pool(name="psum_av", bufs=1, space="PSUM"))

        # ---------------- constants ----------------
        ident = singles.tile([128, 128], DT_MM)
        nc.sync.dma_start(ident, io["ident"].ap())
        neg_t = singles.tile([128, 512], F32)
        nc.vector.memset(neg_t, -70000.0)
        rwb_t = singles.tile([128, NP], F32)
        nc.sync.dma_start(rwb_t, io["rwb_p"].ap())
        rrb_t = singles.tile([128, NP], F32)
        nc.sync.dma_start(rrb_t, io["rrb_p"].ap())

        qbT = resid.tile([128, NP, T], DT_MM)
        q2T = resid.tile([128, NP, T], DT_MM)
        vecT = resid.tile([128, NP, T], DT_MM)

        # bd tails [2048, BDW) are read by boundary chunks (always masked
        # positions) but never written by the BD pass: zero them once.
        zero_bd = singles.tile([128, 512], DT_BD)
        nc.vector.memset(zero_bd, 0.0)
        for buf in io["bd"]:
            nc.sync.dma_start(buf.ap()[:, 2048:2560], zero_bd)

        # ------------- phase A: transposes + projections -------------
        def transpose_half(src, half):
            """src [C, DM] rows half*1024..+1024 -> [128(dm), 8(dmc), 1024(C)]."""
            xT = catT_p.tile([128, 8, 1024], DT_MM, tag="catT")
            for ctg in range(2):          # 512-row groups within the half
                for dmh in range(2):      # 512-col (dm) halves
                    rtiles = []
                    for ct in range(4):
                        row = rows_p.tile([128, 512], DT_MM, tag="rows")
                        r0 = half * 1024 + ctg * 512 + ct * 128
                        nc.sync.dma_start(
                            row, src.ap()[r0:r0 + 128, dmh * 512:(dmh + 1) * 512])
                        rtiles.append(row)
                    for dml in range(4):
                        dmc = dmh * 4 + dml
                        ps = psum_tp.tile([128, 512], DT_MM, tag="tp")
                        for ct in range(4):
                            nc.tensor.transpose(
                                (ps[:, ct * 128:(ct + 1) * 128]),
                                (rtiles[ct][:, dml * 128:(dml + 1) * 128]),
                                (ident),
                            )
                        nc.scalar.copy(xT[:, dmc, ctg * 512:(ctg + 1) * 512], ps)
            return xT

        def load_wset(wname, p):
            ws = wset_p.tile([128, 8, 128], DT_MM, tag="wset")
            nc.sync.dma_start(
                ws,
                io[wname].ap()[:, p * 128:(p + 1) * 128].rearrange(
                    "(o pp) n -> pp o n", pp=128),
            )
            return ws

        wv_t = big_p.tile([128, 8, 512], DT_MM, tag="bigA")
        nc.sync.dma_start(wv_t, io["Wv"].ap().rearrange("(o pp) n -> pp o n", pp=128))

        for half in range(2):
            catT = transpose_half(io["cat"], half)
            # kT
            for p in range(NP):
                ws = load_wset("Wk", p)
                for ch in range(2):
                    cchunk = half * 2 + ch
                    ps = psum_mm.tile([128, 512], F32, tag="mm")
                    for dmc in range(8):
                        nc.tensor.matmul(
                            ps, (ws[:, dmc, :]), (catT[:, dmc, ch * 512:(ch + 1) * 512]),
                            start=(dmc == 0), stop=(dmc == 7),
                        )
                    st = st_p.tile([128, 512], DT_MM, tag="st")
                    nc.scalar.copy(st, ps)
                    nc.sync.dma_start(
                        io["kT_s"].ap()[p, :, cchunk * 512:(cchunk + 1) * 512], st)
            # v
            for cc in range(8):
                ps = psum_mm.tile([128, 512], F32, tag="mm")
                for dmc in range(8):
                    nc.tensor.matmul(
                        ps, (catT[:, dmc, cc * 128:(cc + 1) * 128]), (wv_t[:, dmc, :]),
                        start=(dmc == 0), stop=(dmc == 7),
                    )
                st = st_p.tile([128, 512], DT_MM, tag="st")
                nc.scalar.copy(st, ps)
                nc.sync.dma_start(
                    io["v_s"].ap()[half * 1024 + cc * 128: half * 1024 + (cc + 1) * 128, :], st)
            # q (cat columns >= MEM live in half 1)
            if half == 1:
                for p in range(NP):
                    ws = load_wset("Wq", p)
                    for ih in range(2):
                        ps = psum_mm.tile([128, 512], F32, tag="mm")
                        for dmc in range(8):
                            nc.tensor.matmul(
                                ps, (ws[:, dmc, :]), (catT[:, dmc, ih * 512:(ih + 1) * 512]),
                                start=(dmc == 0), stop=(dmc == 7),
                            )
                        nc.vector.tensor_scalar(
                            qbT[:, p, ih * 512:(ih + 1) * 512], ps,
                            rwb_t[:, p:p + 1], SCALE, ADD, MULT)
                        nc.vector.tensor_scalar(
                            q2T[:, p, ih * 512:(ih + 1) * 512], ps,
                            rrb_t[:, p:p + 1], SCALE, ADD, MULT)

        for half in range(2):
            rT = transpose_half(io["r"], half)
            for p in range(NP):
                ws = load_wset("Wr", p)
                for ch in range(2):
                    cchunk = half * 2 + ch
                    ps = psum_mm.tile([128, 512], F32, tag="mm")
                    for dmc in range(8):
                        nc.tensor.matmul(
                            ps, (ws[:, dmc, :]), (rT[:, dmc, ch * 512:(ch + 1) * 512]),
                            start=(dmc == 0), stop=(dmc == 7),
                        )
                    st = st_p.tile([128, 512], DT_MM, tag="st")
                    nc.scalar.copy(st, ps)
                    nc.sync.dma_start(
                        io["rk_s"].ap()[p, :, cchunk * 512:(cchunk + 1) * 512], st)

        # ------------- phase B: attention -------------
        for p in range(NP):
            kT_t = kpair_p.tile([128, C], DT_MM, tag="kT")
            nc.sync.dma_start(kT_t, io["kT_s"].ap()[p])
            rk_t = kpair_p.tile([128, C], DT_MM, tag="rk")
            nc.sync.dma_start(rk_t, io["rk_s"].ap()[p])
            for sub in range(2):
                hh = 2 * p + sub
                lo, hi = 64 * sub, 64 * sub + 64
                v_t = vhead_p.tile([128, 16, 64], DT_MM, tag="vhead")
                nc.sync.dma_start(
                    v_t,
                    io["v_s"].ap()[:, hh * 64:(hh + 1) * 64].rearrange(
                        "(cc pp) d -> pp cc d", pp=128),
                )

                # BD pass
                for it in range(8):
                    buf = io["bd"][(hh * 8 + it) % NBD]
                    for a in _mchunks(it):
                        off = max(0, _mlo(it) - 512 * a)  # clip to read window
                        w = 512 - off
                        ps = psum_mm.tile([128, 512], F32, tag="mm")
                        nc.tensor.matmul(
                            ps[:, :w],
                            (q2T[lo:hi, p, it * 128:(it + 1) * 128]),
                            (rk_t[lo:hi, a * 512 + off:(a + 1) * 512]),
                            start=True, stop=True,
                        )
                        st = st_p.tile([128, 512], DT_BD, tag="bdst")
                        if (it + a) % 2 == 0:
                            nc.scalar.copy(st[:, :w], ps[:, :w])
                        else:
                            nc.vector.tensor_copy(st[:, :w], ps[:, :w])
                        nc.sync.dma_start(
                            buf.ap()[:, a * 512 + off:(a + 1) * 512], st[:, :w])

                denoms = den_p.tile([128, 8, 4], F32, tag="denoms")
                recips = den_p.tile([128, 8], F32, tag="recips")

                # scores -> exp -> P^T, per i-half
                for ihalf in range(2):
                    njc = 12 if ihalf == 0 else 16
                    PTa = big_p.tile([128, 8, 512], DT_MM, tag="bigA")
                    PTb = big_p.tile([128, 8, 512], DT_MM, tag="bigB")

                    def PTs(jc):
                        return (PTa, jc) if jc < 8 else (PTb, jc - 8)
                    for itl in range(4):
                        it = ihalf * 4 + itl
                        buf = io["bd"][(hh * 8 + it) % NBD]
                        cm = _cmax(it)
                        for c in range(cm + 1):
                            wb = _wb(it) if c == cm else 512
                            ps = psum_mm.tile([128, 512], F32, tag="mm")
                            nc.tensor.matmul(
                                ps,
                                (qbT[lo:hi, p, it * 128:(it + 1) * 128]),
                                (kT_t[lo:hi, c * 512:(c + 1) * 512]),
                                start=True, stop=True,
                            )
                            skew = skew_p.tile([128, 512], DT_BD, tag="skew")
                            nc.sync.dma_start(
                                skew[:, :wb],
                                bass.AP(buf, 512 * c + (T - 1) - it * 128,
                                        [[BDW - 1, 128], [1, wb]]),
                            )
                            s_t = sch_p.tile([128, 512], F32, tag="S")
                            nc.vector.tensor_tensor(
                                s_t[:, :wb], ps[:, :wb], skew[:, :wb], ADD)
                            if c == cm:
                                # boundary chunk: push masked scores to -inf
                                mk = mask_p.tile([128, 512], mybir.dt.uint8, tag="mask")
                                nc.sync.dma_start(
                                    mk, io["masku8"].ap()[
                                        it * 128:(it + 1) * 128, cm * 512:(cm + 1) * 512])
                                nc.vector.copy_predicated(s_t, mk, neg_t)
                            P_c = pch_p.tile([128, 512], DT_MM, tag="P")
                            nc.scalar.activation(
                                P_c, s_t, mybir.ActivationFunctionType.Exp,
                                accum_out=denoms[:, it, c:c + 1],
                            )
                            # transpose the 4 jc blocks of this chunk into PT
                            tps = psum_tp.tile([128, 512], DT_MM, tag="tp")
                            for j4 in range(4):
                                nc.tensor.transpose(
                                    (tps[:, j4 * 128:(j4 + 1) * 128]),
                                    (P_c[:, j4 * 128:(j4 + 1) * 128]),
                                    (ident),
                                )
                            pt_t, jb = PTs(c * 4)
                            dst = pt_t[:, jb:jb + 4, itl * 128:(itl + 1) * 128]
                            src = tps.rearrange("p (a b) -> p a b", a=4)
                            if it % 2 == 0:
                                nc.scalar.copy(dst, src)
                            else:
                                nc.vector.tensor_copy(dst, src)
                        nc.vector.tensor_reduce(
                            recips[:, it:it + 1], denoms[:, it, 0:cm + 1],
                            axis=mybir.AxisListType.X, op=ADD,
                        )
                    # reciprocals for this i-half -> DRAM (re-read broadcast below)
                    hsl = slice(ihalf * 4, (ihalf + 1) * 4)
                    nc.vector.reciprocal(recips[:, hsl], recips[:, hsl])
                    nc.sync.dma_start(
                        bass.AP(io["recip_s"], hh * T + ihalf * 512, [[1, 128], [128, 4]]),
                        recips[:, hsl])
                    av = psum_av.tile([64, 512], F32, tag="av")
                    for jc in range(njc):
                        pt_t, jb = PTs(jc)
                        nc.tensor.matmul(
                            av,
                            (v_t[:, jc, :]),
                            (pt_t[:, jb, :]),
                            start=(jc == 0), stop=(jc == njc - 1),
                        )
                    rb = rb_p.tile([64, 512], F32, tag="rb")
                    nc.sync.dma_start(
                        rb,
                        bass.AP(io["recip_s"], hh * T + ihalf * 512, [[0, 64], [1, 512]]))
                    if sub == 0:
                        nc.vector.tensor_tensor(
                            vecT[0:64, p, ihalf * 512:(ihalf + 1) * 512], av, rb, MULT)
                    else:
                        # odd head: epilogue at base 0, partition-shift via DMA
                        tmp = rb_p.tile([64, 512], DT_MM, tag="avtmp")
                        nc.vector.tensor_tensor(tmp, av, rb, MULT)
                        nc.sync.dma_start(
                            vecT[64:128, p, ihalf * 512:(ihalf + 1) * 512], tmp)

        # ------------- phase C: output projection -------------
        for dmc in range(2):
            for itg in range(2):
                pss = [psum_mm.tile([128, 512], F32, tag="mm", name=f"wo_ps{i}")
                       for i in range(4)]
                for pp in range(NP):
                    wt = wo_p.tile([128, 512], DT_MM, tag="wo")
                    nc.sync.dma_start(
                        wt, io["Wo"].ap()[pp * 128:(pp + 1) * 128, dmc * 512:(dmc + 1) * 512])
                    for itl in range(4):
                        it = itg * 4 + itl
                        nc.tensor.matmul(
                            pss[itl], (vecT[:, pp, it * 128:(it + 1) * 128]), (wt),
                            start=(pp == 0), stop=(pp == NP - 1),
                        )
                for itl in range(4):
                    it = itg * 4 + itl
                    st = st_p.tile([128, 512], F32, tag="st")
                    nc.scalar.copy(st, pss[itl])
                    nc.sync.dma_start(
                        io["out"].ap()[it * 128:(it + 1) * 128, dmc * 512:(dmc + 1) * 512], st)


_NC = None


def _get_nc():
    global _NC
    if _NC is None:
        _NC = build_nc()
    return _NC


def make_in_maps(h, m, r, mask, W_qkv, W_r, W_o, r_w_bias, r_r_bias):
    h = np.ascontiguousarray(np.asarray(h, dtype=np.float32))
    m = np.ascontiguousarray(np.asarray(m, dtype=np.float32))
    r = np.ascontiguousarray(np.asarray(r, dtype=np.float32))
    masku8 = np.ascontiguousarray(np.asarray(mask).reshape(T, C).astype(np.uint8))
    W_qkv = np.asarray(W_qkv, dtype=np.float32)
    W_r = np.asarray(W_r, dtype=np.float32)
    W_o = np.asarray(W_o, dtype=np.float32)
    rwb = np.asarray(r_w_bias, dtype=np.float32)
    rrb = np.asarray(r_r_bias, dtype=np.float32)

    in_maps = []
    for core in range(8):
        b, nh = core // 2, core % 2
        sl = slice(nh * NH * D, (nh + 1) * NH * D)
        rwb_p = np.zeros((128, NP), np.float32)
        rrb_p = np.zeros((128, NP), np.float32)
        for hh in range(NH):
            g = nh * NH + hh
            rwb_p[64 * (hh % 2):64 * (hh % 2) + 64, hh // 2] = rwb[g]
            rrb_p[64 * (hh % 2):64 * (hh % 2) + 64, hh // 2] = rrb[g]
        in_maps.append({
            "cat": np.ascontiguousarray(np.concatenate([m[:, b, :], h[:, b, :]], axis=0)),
            "r": r,
            "Wq": np.ascontiguousarray(W_qkv[:, 0 * N * D:1 * N * D][:, sl]),
            "Wk": np.ascontiguousarray(W_qkv[:, 1 * N * D:2 * N * D][:, sl]),
            "Wv": np.ascontiguousarray(W_qkv[:, 2 * N * D:3 * N * D][:, sl]),
            "Wr": np.ascontiguousarray(W_r[:, sl]),
            "Wo": np.ascontiguousarray(W_o[sl, :]),
            "rwb_p": rwb_p,
            "rrb_p": rrb_p,
            "masku8": masku8,
            "ident": np.eye(128, dtype=np.float32),
        })
    return in_maps


def finish(h, parts, ln_gamma, ln_beta):
    h = np.asarray(h, dtype=np.float32)
    gamma = np.asarray(ln_gamma, dtype=np.float32)
    beta = np.asarray(ln_beta, dtype=np.float32)
    out = np.empty((T, B, DM), np.float32)
    for b in range(B):
        x = h[:, b, :] + parts[2 * b] + parts[2 * b + 1]
        mu = x.mean(axis=-1, keepdims=True, dtype=np.float32)
        var = ((x - mu) ** 2).mean(axis=-1, keepdims=True, dtype=np.float32)
        out[:, b, :] = (x - mu) / np.sqrt(var + LN_EPS) * gamma + beta
    return out


def kernel(h, m, r, mask, W_qkv, W_r, W_o, r_w_bias, r_r_bias, ln_gamma, ln_beta):
    from concourse.bass_utils import run_bass_kernel_spmd

    in_maps = make_in_maps(h, m, r, mask, W_qkv, W_r, W_o, r_w_bias, r_r_bias)
    res = run_bass_kernel_spmd(_get_nc(), in_maps, core_ids=list(range(8)))
    parts = [np.asarray(res.results[c]["out"]) for c in range(8)]
    return finish(h, parts, ln_gamma, ln_beta)
